# revision 6
# baseline (speedup 1.0000x reference)
"""Trainium2 Bass kernel for nn_MultiDirectionPattern (segment_reduce).

Computes, for x [B=256, C=2048, 7, 7] and s [B, 1, 56, 56]:
  feat[b,c,p]  = sum over 7x7 cells in direction-sector p of x, / count(p)
  sal[b,n,p]   = s.flat[b, sal_idx[p, n]]            (static gather)

Sharding: pure data parallelism over B across 8 NeuronCores (32 b/core).

Per-core device strategy (memory-bound, no matmul, no transpose):
  * x rows (b,c) live on SBUF partitions (128), 512 rows per partition,
    processed in UNITS chunks; every DMA moves large contiguous
    per-partition blocks.
  * Each direction sector of the 7x7 mask decomposes as
    center + ray_a(3 cells, arithmetic stride) + ray_b + 3 interior cells;
    ray sums are shared by adjacent sectors.  All sums are fp32 DVE
    tensor_tensor adds on strided access-pattern views — the 49-cell axis
    never needs to be transposed onto partitions.  Ops are fused pairwise
    (two sectors / two rays per instruction) via a 2-count AP dim, since
    any two offsets form a valid access pattern.
  * The final (z*1/count + center/count) is one fused scalar_tensor_tensor.
  * The saliency gather: each sector's flat indices are 28 contiguous runs
    (triangles), exactly 8 runs of each length 1..28.  s is DMAed to SBUF
    once; same-length run pairs are gathered by fused ScalarE copies
    straight into the n-major interleaved output layout.
"""

import sys

for _p in ("/opt/trn_rl_repo",):
    if _p not in sys.path:
        sys.path.append(_p)

import numpy as np

import bass_rust
import concourse.bass as bass
import concourse.mybir as mybir
import concourse.tile as tile
from concourse.bass_utils import run_bass_kernel_spmd

F32 = mybir.dt.float32

N_CORES = 8
B, C, HW = 256, 2048, 49
B_LOC = B // N_CORES              # 32 batch rows per core
ROWS = B_LOC * C                  # 65536 x-rows per core
P = 128                           # SBUF partitions
RPP = ROWS // P                   # 512 rows per partition
UNITS = 2
R = RPP // UNITS                  # rows per partition per unit
S_FLAT = 56 * 56                  # 3136
N_SAL = 406
SAL_W = 8 * N_SAL                 # 3248

ADD = mybir.AluOpType.add
MULT = mybir.AluOpType.mult


def _ap(t_ap: bass.AP, offset: int, dims) -> bass.AP:
    """Raw AP over the tensor behind `t_ap`: dims = [(step, count), ...]."""
    return bass.AP(t_ap.tensor, offset, [[s, c] for s, c in dims])


def _split_excess_waits(nc: bass.Bass) -> None:
    """Move excess per-instruction sem waits onto injected same-engine NoOps.

    This walrus build enforces tight per-instruction sync-wait capacity
    (1 for most opcodes, 2 for EventSemaphore), but Tile can attach more
    (e.g. data dep + DMA ring-head wait on one DMACopy, or the exit drain's
    one-wait-per-outstanding-proc).  Waits execute sequentially in the
    engine's stream, so hoisting all but the last onto preceding NoOps is
    semantics-preserving.
    """
    n = 0
    for f in nc.m.functions:
        for bb in f.blocks:
            insts = bb.instructions
            i = 0
            while i < len(insts):
                inst = insts[i]
                si = inst.sync_info
                cap = 2 if isinstance(inst, mybir.InstEventSemaphore) else 1
                if si is not None and len(si.on_wait) > cap:
                    waits = list(si.on_wait)
                    for w in waits[:-cap]:
                        nop = mybir.InstNoOp(
                            name=f"waitsplit-{n}", engine=inst.engine
                        )
                        n += 1
                        nop.sync_info = bass_rust.SyncInfo(
                            on_wait=[w], on_update=[]
                        )
                        insts.insert(i, nop)
                        i += 1
                    inst.sync_info = bass_rust.SyncInfo(
                        on_wait=waits[-cap:], on_update=list(si.on_update)
                    )
                i += 1


def _feat_tables(feat_mask: np.ndarray):
    """Decompose each 7x7 sector into center + two 3-cell rays + 3 interior
    cells."""
    m = feat_mask.reshape(8, HW)
    assert np.all((m == 0.0) | (m == 1.0)), "feat_mask must be binary"
    counts = (m != 0).sum(axis=1)
    assert np.all(counts == counts[0]), "expect uniform sector counts"
    center = HW // 2
    ray_deltas = [1, -1, 7, -7, 8, -8, 6, -6]
    rays = [tuple(center + d * t for t in (1, 2, 3)) for d in ray_deltas]
    ray_sets = [frozenset(r) for r in rays]
    dirs = []
    for d in range(8):
        cells = set(np.flatnonzero(m[d]).tolist())
        assert center in cells
        rest = cells - {center}
        hit = [k for k, rs in enumerate(ray_sets) if rs <= rest]
        assert len(hit) == 2, f"dir {d}: sector is not center+2 rays ({hit})"
        interior = sorted(rest - ray_sets[hit[0]] - ray_sets[hit[1]])
        assert len(interior) == 3, f"dir {d}: interior {interior}"
        dirs.append((hit[0], hit[1], tuple(interior)))
    used = sorted({k for a, b, _ in dirs for k in (a, b)})
    remap = {k: i for i, k in enumerate(used)}
    rays_used = [rays[k] for k in used]
    dirs = [(remap[a], remap[b], i) for a, b, i in dirs]
    return rays_used, dirs, float(1.0 / counts[0]), center


def _sal_runs(sal_idx: np.ndarray):
    """[(dir, out_start, src_start, length)] — each sector's ascending flat
    indices split into maximal consecutive runs."""
    runs = []
    for d in range(8):
        idx = sal_idx[d]
        n0 = 0
        start = int(idx[0])
        prev = start
        for n in range(1, len(idx)):
            v = int(idx[n])
            if v != prev + 1:
                runs.append((d, n0, start, n - n0))
                n0, start = n, v
            prev = v
        runs.append((d, n0, start, len(idx) - n0))
    return runs


def _build(feat_mask: np.ndarray, sal_idx: np.ndarray, split_waits: bool = True) -> bass.Bass:
    rays, dirs, inv_cnt, center = _feat_tables(feat_mask)
    runs = _sal_runs(sal_idx)

    nc = bass.Bass()
    x = nc.declare_dram_parameter("x", [ROWS, HW], F32, isOutput=False)
    s = nc.declare_dram_parameter("s", [B_LOC, S_FLAT], F32, isOutput=False)
    feat = nc.declare_dram_parameter("feat", [ROWS, 8], F32, isOutput=True)
    sal = nc.declare_dram_parameter("sal", [B_LOC, SAL_W], F32, isOutput=True)

    xv = x[:].rearrange("(p u r) c -> p u r c", p=P, u=UNITS)
    fv = feat[:].rearrange("(p u r) c -> p u r c", p=P, u=UNITS)

    # pair up the 8 rays: (+d, -d) pairs already adjacent in `rays` order?
    # Just pair consecutive entries; any two offsets form an AP.
    ray_pairs = [(0, 1), (2, 3), (4, 5), (6, 7)]
    dir_pairs = [(0, 1), (2, 3), (4, 5), (6, 7)]

    # pair same-length saliency runs
    from collections import defaultdict
    runs_by_len = defaultdict(list)
    for r_ in runs:
        runs_by_len[r_[3]].append(r_)
    run_pairs = []
    for ln, group in sorted(runs_by_len.items()):
        for i in range(0, len(group) - 1, 2):
            run_pairs.append((group[i], group[i + 1]))
        if len(group) % 2:
            run_pairs.append((group[-1], None))

    with tile.TileContext(nc) as tc:
        with (
            tc.tile_pool(name="xin", bufs=2) as xin_pool,
            tc.tile_pool(name="fout", bufs=2) as fout_pool,
            tc.tile_pool(name="tmp", bufs=2) as tmp_pool,
            tc.tile_pool(name="salp", bufs=1) as sal_pool,
        ):
            # ---- saliency gather on ScalarE (fused same-length run pairs) --
            st = sal_pool.tile([B_LOC, S_FLAT], F32)
            nc.sync.dma_start(st[:], s[:])
            salo = sal_pool.tile([B_LOC, SAL_W], F32)
            sps = S_FLAT   # st partition stride (elements)
            ops = SAL_W    # salo partition stride
            for ra, rb in run_pairs:
                d1, n1, s1, ln = ra
                if rb is None:
                    src = _ap(st[:], s1, [(sps, B_LOC), (1, ln)])
                    dst = _ap(salo[:], n1 * 8 + d1, [(ops, B_LOC), (8, ln)])
                else:
                    d2, n2, s2, _ = rb
                    src = _ap(st[:], s1,
                              [(sps, B_LOC), (s2 - s1, 2), (1, ln)])
                    dst = _ap(salo[:], n1 * 8 + d1,
                              [(ops, B_LOC),
                               ((n2 * 8 + d2) - (n1 * 8 + d1), 2), (8, ln)])
                nc.scalar.copy(dst, src)
            nc.sync.dma_start(sal[:], salo[:])

            # ---- directional mean-pool over x (fused pairs on DVE) ----
            xps = R * HW   # xt partition stride
            for u in range(UNITS):
                xt = xin_pool.tile([P, R, HW], F32)
                nc.sync.dma_start(xt[:], xv[:, u])
                xta = xt[:]

                def xcells(ca, cb):
                    # [128, 2, R]: cell ca for slot0, cb for slot1
                    return _ap(xta, ca, [(xps, P), (cb - ca, 2), (HW, R)])

                ft = fout_pool.tile([P, R, 8], F32)
                fta = ft[:]
                rays_t = tmp_pool.tile([P, 8, R], F32)
                ra = rays_t[:]
                pq_t = tmp_pool.tile([P, 8, R], F32)
                pa = pq_t[:]
                wz_t = tmp_pool.tile([P, 8, R], F32)
                wa = wz_t[:]

                def slot2(t_ap, pstride, ka, kb, inner_step=1, base_mul=None):
                    bm = pstride // 8 if base_mul is None else base_mul
                    return _ap(t_ap, ka * bm,
                               [(pstride, P), ((kb - ka) * bm, 2),
                                (inner_step, R)])

                cc = tmp_pool.tile([P, R], F32)
                nc.vector.tensor_scalar_mul(cc[:], xt[:, :, center], inv_cnt)

                # ray sums (pairs of rays per op)
                for ka, kb in ray_pairs:
                    A, Bc = rays[ka], rays[kb]
                    out2 = slot2(ra, 8 * R, ka, kb, 1, R)
                    nc.vector.tensor_tensor(
                        out2, xcells(A[0], Bc[0]), xcells(A[1], Bc[1]), ADD)
                    nc.vector.tensor_tensor(
                        out2, out2, xcells(A[2], Bc[2]), ADD)

                # per-dir interiors + combine (pairs of dirs per op)
                for da, db in dir_pairs:
                    ka1, ka2, ia = dirs[da]
                    kb1, kb2, ib = dirs[db]
                    pq2 = slot2(pa, 8 * R, da, db, 1, R)
                    nc.vector.tensor_tensor(
                        pq2, xcells(ia[0], ib[0]), xcells(ia[1], ib[1]), ADD)
                    nc.vector.tensor_tensor(
                        pq2, pq2, xcells(ia[2], ib[2]), ADD)
                    wz2 = slot2(wa, 8 * R, da, db, 1, R)
                    nc.vector.tensor_tensor(
                        wz2, slot2(ra, 8 * R, ka1, kb1, 1, R),
                        slot2(ra, 8 * R, ka2, kb2, 1, R), ADD)
                    nc.vector.tensor_tensor(wz2, wz2, pq2, ADD)
                    # feat = wz * inv_cnt + cc   (cc broadcast over the pair)
                    ft2 = _ap(fta, da, [(8 * R, P), (db - da, 2), (8, R)])
                    cc2 = _ap(cc[:], 0, [(R, P), (0, 2), (1, R)])
                    nc.vector.scalar_tensor_tensor(
                        ft2, wz2, inv_cnt, cc2, op0=MULT, op1=ADD)

                nc.sync.dma_start(fv[:, u], ft[:])

    if split_waits:
        _split_excess_waits(nc)
    return nc


_CACHE: dict = {}


def _get_nc(feat_mask: np.ndarray, sal_idx: np.ndarray) -> bass.Bass:
    key = (feat_mask.tobytes(), sal_idx.tobytes())
    if key not in _CACHE:
        _CACHE[key] = _build(feat_mask, sal_idx)
    return _CACHE[key]


def make_in_maps(x, s):
    x = np.ascontiguousarray(np.asarray(x, dtype=np.float32))
    s = np.ascontiguousarray(np.asarray(s, dtype=np.float32))
    in_maps = []
    for core in range(N_CORES):
        b0 = core * B_LOC
        in_maps.append({
            "x": x[b0:b0 + B_LOC].reshape(ROWS, HW),
            "s": s[b0:b0 + B_LOC].reshape(B_LOC, S_FLAT),
        })
    return in_maps


def assemble(results):
    feat = np.concatenate([r["feat"] for r in results], axis=0)
    sal = np.concatenate([r["sal"] for r in results], axis=0)
    return (
        feat.reshape(B, C, 2, 4),
        sal.reshape(B, N_SAL, 2, 4),
    )


def kernel(x, s, feat_mask, sal_idx):
    feat_mask = np.asarray(feat_mask, dtype=np.float32)
    sal_idx = np.asarray(sal_idx)
    nc = _get_nc(feat_mask, sal_idx)
    res = run_bass_kernel_spmd(nc, make_in_maps(x, s), list(range(N_CORES)))
    return assemble(res.results)
